# revision 4
# baseline (speedup 1.0000x reference)
"""Trainium2 Bass kernel for BatchedSimpleEIColumns.

Column-parallel sharding: C=512 columns split across 8 NeuronCores (64 each).
Per column c the update is

    I_e = (thal+inc_c) @ IP_c.T + l23_c @ FP_c.T + re_c @ Wee_c.T - ri_c @ Wie_c.T
    I_i = re_c @ Wei_c.T
    v_e' = 0.9*e_v + 0.1*I_e ; r_e' = relu(v_e')   (same for i-side)

All contractions run over the last (contiguous) dim of both operands, so both
matmul operands are brought K-major on-chip via PE transposes; the 0.1 factor
is folded into the transposed-weight copies, giving a single fused
scalar_tensor_tensor (v = 0.9*state + psum) per output tile.
"""

import numpy as np
from contextlib import ExitStack

import concourse.bass as bass
import concourse.tile as tile
from concourse import bacc, mybir
from concourse.alu_op_type import AluOpType
from concourse.bass_utils import run_bass_kernel_spmd

f32 = mybir.dt.float32
bf16 = mybir.dt.bfloat16

B, C, NE, NI, E = 64, 512, 80, 20, 512
CORES = 8
DT = 0.1

# matmul operand dtype: bf16 halves PE time and SBUF footprint (inputs are
# cast for free during the SWDGE DMA); fp32 is the exact fallback.
MM_DT = bf16


def build(cl=C // CORES, cb=8, md=MM_DT, n_cores=CORES,
          ldx_bufs=2, ldw_bufs=2, kt_bufs=4, ps_bufs=4, mm_bufs=3,
          wtip_act=False, xts_act=False):
    """Build the per-core Bass program. cl: columns per core, cb: columns per
    load batch, md: matmul operand dtype."""
    assert cl % cb == 0
    nc = bacc.Bacc("TRN2", target_bir_lowering=False, debug=False,
                   num_devices=n_cores)

    di = {}
    for name, shp in [
        ("thal", [B, E]), ("thal_increments", [B, cl, E]),
        ("l23_fb", [B, cl, NE]), ("r_e", [B, cl, NE]), ("r_i", [B, cl, NI]),
        ("e_v", [B, cl, NE]), ("i_v", [B, cl, NI]),
        ("input_proj", [cl, NE, E]), ("feedback_proj", [cl, NE, NE]),
        ("W_ee", [cl, NE, NE]), ("W_ei", [cl, NI, NE]), ("W_ie", [cl, NE, NI]),
    ]:
        di[name] = nc.dram_tensor(name, shp, f32, kind="ExternalInput")
    do = {}
    for name, shp in [
        ("r_e_new", [B, cl, NE]), ("r_i_new", [B, cl, NI]),
        ("v_e_new", [B, cl, NE]), ("v_i_new", [B, cl, NI]),
    ]:
        do[name] = nc.dram_tensor(name, shp, f32, kind="ExternalOutput")

    ident_d = nc.inline_tensor(np.eye(128, dtype=np.float32), name="ident_f32")

    def cast_load(eng_t, dram_ap):
        # SWDGE casts f32->bf16 during the transfer; HWDGE for plain f32.
        if eng_t.dtype == f32:
            nc.sync.dma_start(eng_t, dram_ap)
        else:
            nc.gpsimd.dma_start(eng_t, dram_ap)

    with tile.TileContext(nc) as tc, ExitStack() as ctx:
        const_p = ctx.enter_context(tc.tile_pool(name="const", bufs=1))
        ldx = ctx.enter_context(tc.tile_pool(name="ldx", bufs=ldx_bufs))
        ldw = ctx.enter_context(tc.tile_pool(name="ldw", bufs=ldw_bufs))
        kt = ctx.enter_context(tc.tile_pool(name="kt", bufs=kt_bufs))
        outp = ctx.enter_context(tc.tile_pool(name="outp", bufs=2))
        ps = ctx.enter_context(tc.tile_pool(name="ps", bufs=ps_bufs, space="PSUM"))
        mmps = ctx.enter_context(tc.tile_pool(name="mmps", bufs=mm_bufs, space="PSUM"))

        identf = const_p.tile([128, 128], f32)
        nc.sync.dma_start(identf[:], ident_d[:])
        if md != f32:
            ident = const_p.tile([128, 128], md)
            nc.vector.tensor_copy(ident[:], identf[:])
        else:
            ident = identf

        # thal [B, E] loaded once, transposed once -> thalT [128, 4*B] md
        thal_t = const_p.tile([B, E], md)
        cast_load(thal_t[:], di["thal"][:])
        thalT = const_p.tile([128, 4 * B], md)
        ptt = ps.tile([128, 4 * B], md, tag="pt")
        for k in range(4):
            nc.tensor.transpose(ptt[:, k * B:(k + 1) * B],
                                thal_t[:, k * 128:(k + 1) * 128],
                                ident[0:B, 0:B])
        nc.vector.tensor_copy(thalT[:], ptt[:])

        for b0 in range(0, cl, cb):
            csl = slice(b0, b0 + cb)
            inc_t = ldx.tile([B, cb, E], md, tag="inc")
            cast_load(inc_t[:], di["thal_increments"][:, csl, :])
            l23_t = ldx.tile([B, cb, NE], md, tag="l23")
            cast_load(l23_t[:], di["l23_fb"][:, csl, :])
            re_t = ldx.tile([B, cb, NE], md, tag="re")
            cast_load(re_t[:], di["r_e"][:, csl, :])
            ri_t = ldx.tile([B, cb, NI], md, tag="ri")
            cast_load(ri_t[:], di["r_i"][:, csl, :])
            ev_t = ldx.tile([B, cb, NE], f32, tag="ev")
            nc.sync.dma_start(ev_t[:], di["e_v"][:, csl, :])
            iv_t = ldx.tile([B, cb, NI], f32, tag="iv")
            nc.sync.dma_start(iv_t[:], di["i_v"][:, csl, :])

            # weights, o-major: [o, c, k]
            ip_t = ldw.tile([NE, cb, E], md, tag="ip")
            cast_load(ip_t[:], di["input_proj"][csl].rearrange("c o e -> o c e"))
            fp_t = ldw.tile([NE, cb, NE], md, tag="fp")
            cast_load(fp_t[:], di["feedback_proj"][csl].rearrange("c o e -> o c e"))
            wee_t = ldw.tile([NE, cb, NE], md, tag="wee")
            cast_load(wee_t[:], di["W_ee"][csl].rearrange("c o e -> o c e"))
            wei_t = ldw.tile([NI, cb, NE], md, tag="wei")
            cast_load(wei_t[:], di["W_ei"][csl].rearrange("c o e -> o c e"))
            wie_t = ldw.tile([NE, cb, NI], md, tag="wie")
            cast_load(wie_t[:], di["W_ie"][csl].rearrange("c o e -> o c e"))

            ven_t = outp.tile([B, cb, NE], f32, tag="ven")
            ren_t = outp.tile([B, cb, NE], f32, tag="ren")
            vin_t = outp.tile([B, cb, NI], f32, tag="vin")
            rin_t = outp.tile([B, cb, NI], f32, tag="rin")

            for j in range(cb):
                # --- K-major activations ---
                XT_e = kt.tile([128, 4 * B], md, tag="xte")
                pte = ps.tile([128, 4 * B], md, tag="pt")
                for k in range(4):
                    nc.tensor.transpose(pte[:, k * B:(k + 1) * B],
                                        inc_t[:, j, k * 128:(k + 1) * 128],
                                        ident[0:B, 0:B])
                nc.vector.tensor_add(XT_e[:], pte[:], thalT[:])

                XT_s = kt.tile([NE, 3 * B], md, tag="xts")
                pts = ps.tile([NE, 3 * B], md, tag="pt")
                nc.tensor.transpose(pts[0:NE, 0:B], l23_t[:, j, :], ident[0:B, 0:B])
                nc.tensor.transpose(pts[0:NE, B:2 * B], re_t[:, j, :], ident[0:B, 0:B])
                nc.tensor.transpose(pts[0:NI, 2 * B:3 * B], ri_t[:, j, :], ident[0:B, 0:B])
                xts_eng = nc.scalar if xts_act else nc.vector
                xts_eng.tensor_copy(XT_s[:, 0:2 * B], pts[:, 0:2 * B])
                xts_eng.tensor_copy(XT_s[0:NI, 2 * B:3 * B], pts[0:NI, 2 * B:3 * B])

                # --- K-major weights, scaled by DT (and -DT for Wie) ---
                WT_ip = kt.tile([128, 4 * NE], md, tag="wtip")
                ptw = ps.tile([128, 4 * NE], md, tag="pt")
                for k in range(4):
                    nc.tensor.transpose(ptw[:, k * NE:(k + 1) * NE],
                                        ip_t[:, j, k * 128:(k + 1) * 128],
                                        ident[0:NE, 0:NE])
                if wtip_act:
                    nc.scalar.mul(WT_ip[:], ptw[:], DT)
                else:
                    nc.vector.tensor_scalar_mul(WT_ip[:], ptw[:], DT)

                WT_s = kt.tile([NE, 260], md, tag="wts")
                ptws = ps.tile([NE, 260], md, tag="pt")
                nc.tensor.transpose(ptws[0:NE, 0:NE], fp_t[:, j, :], ident[0:NE, 0:NE])
                nc.tensor.transpose(ptws[0:NE, NE:2 * NE], wee_t[:, j, :], ident[0:NE, 0:NE])
                nc.tensor.transpose(ptws[0:NE, 2 * NE:2 * NE + NI], wei_t[:, j, :], ident[0:NI, 0:NI])
                nc.tensor.transpose(ptws[0:NI, 180:260], wie_t[:, j, :], ident[0:NE, 0:NE])
                nc.scalar.mul(WT_s[:, 0:180], ptws[:, 0:180], DT)
                nc.scalar.mul(WT_s[0:NI, 180:260], ptws[0:NI, 180:260], -DT)

                # --- matmuls, one accumulation group: cols 0:80 = 0.1*I_e,
                # cols 80:100 = 0.1*I_i (disjoint columns, same bank) ---
                pe = mmps.tile([B, NE + NI], f32, tag="pe")
                for k in range(4):
                    nc.tensor.matmul(pe[:, 0:NE], XT_e[:, k * B:(k + 1) * B],
                                     WT_ip[:, k * NE:(k + 1) * NE],
                                     start=(k == 0), stop=False)
                nc.tensor.matmul(pe[:, 0:NE], XT_s[0:NE, 0:B], WT_s[0:NE, 0:NE],
                                 start=False, stop=False)
                nc.tensor.matmul(pe[:, 0:NE], XT_s[0:NE, B:2 * B], WT_s[0:NE, NE:2 * NE],
                                 start=False, stop=False)
                nc.tensor.matmul(pe[:, 0:NE], XT_s[0:NI, 2 * B:3 * B], WT_s[0:NI, 180:260],
                                 start=False, stop=False)
                nc.tensor.matmul(pe[:, NE:NE + NI], XT_s[0:NE, B:2 * B],
                                 WT_s[0:NE, 2 * NE:2 * NE + NI],
                                 start=False, stop=True)

                # --- leaky integration ---
                nc.vector.scalar_tensor_tensor(ven_t[:, j, :], ev_t[:, j, :], 1.0 - DT,
                                               pe[:, 0:NE], AluOpType.mult, AluOpType.add)
                nc.vector.scalar_tensor_tensor(vin_t[:, j, :], iv_t[:, j, :], 1.0 - DT,
                                               pe[:, NE:NE + NI], AluOpType.mult, AluOpType.add)

            nc.scalar.activation(ren_t[:], ven_t[:], mybir.ActivationFunctionType.Relu)
            nc.scalar.activation(rin_t[:], vin_t[:], mybir.ActivationFunctionType.Relu)

            nc.sync.dma_start(do["v_e_new"][:, csl, :], ven_t[:])
            nc.sync.dma_start(do["r_e_new"][:, csl, :], ren_t[:])
            nc.sync.dma_start(do["v_i_new"][:, csl, :], vin_t[:])
            nc.sync.dma_start(do["r_i_new"][:, csl, :], rin_t[:])

    nc.compile()
    return nc


_NC_CACHE = {}


def _get_nc():
    key = (C // CORES, 8, MM_DT)
    if key not in _NC_CACHE:
        _NC_CACHE[key] = build(cl=key[0], cb=key[1], md=key[2])
    return _NC_CACHE[key]


def shard_inputs(inputs):
    cl = C // CORES
    maps = []
    for i in range(CORES):
        sl = slice(i * cl, (i + 1) * cl)
        maps.append({
            "thal": np.ascontiguousarray(inputs["thal"]),
            "thal_increments": np.ascontiguousarray(inputs["thal_increments"][:, sl, :]),
            "l23_fb": np.ascontiguousarray(inputs["l23_fb"][:, sl, :]),
            "r_e": np.ascontiguousarray(inputs["r_e"][:, sl, :]),
            "r_i": np.ascontiguousarray(inputs["r_i"][:, sl, :]),
            "e_v": np.ascontiguousarray(inputs["e_v"][:, sl, :]),
            "i_v": np.ascontiguousarray(inputs["i_v"][:, sl, :]),
            "input_proj": np.ascontiguousarray(inputs["input_proj"][sl]),
            "feedback_proj": np.ascontiguousarray(inputs["feedback_proj"][sl]),
            "W_ee": np.ascontiguousarray(inputs["W_ee"][sl]),
            "W_ei": np.ascontiguousarray(inputs["W_ei"][sl]),
            "W_ie": np.ascontiguousarray(inputs["W_ie"][sl]),
        })
    return maps


def unshard_outputs(results):
    outs = []
    for name in ("r_e_new", "r_i_new", "v_e_new", "v_i_new"):
        outs.append(np.concatenate([results[i][name] for i in range(CORES)], axis=1))
    return tuple(outs)


def kernel(**inputs):
    inputs = {k: np.asarray(v, dtype=np.float32) for k, v in inputs.items()}
    nc = _get_nc()
    res = run_bass_kernel_spmd(nc, shard_inputs(inputs), list(range(CORES)))
    return unshard_outputs(res.results)
